# revision 3
# baseline (speedup 1.0000x reference)
"""Trainium2 Bass kernel for ContinuousIntegratedKoopmanOperator.

reference: odeint(dz/dt = z @ W) sampled at t = DT*[1..T], y0 = x at t[0].
Closed form (time-invariant linear ODE): out[:, j, :] = x @ expm(DT*j*W).

Strategy:
  host: compute Mj = expm(DT*j*W) for j=0..T-1 in float64, cast to f32,
        concat to M (128, T*128).
  device (8 cores, batch-sharded 1024 rows each):
        out_tile[r, j*128+d] = sum_k x[r,k] * M[k, j*128+d]
        i.e. 8 batch tiles x 16 j-blocks of (128x128)@(128x512) fp32 matmuls,
        PSUM bank rotation, DVE drain to staging, 4MB contiguous DMA out.
"""
import numpy as np

DT = 0.01
B, D, T = 8192, 128, 64
NCORES = 8
BSH = B // NCORES          # 1024 rows per core
NTILES = BSH // 128        # 8 batch tiles per core
BW = 512                   # j-block width (4 j's of 128)
NBLK = (T * D) // BW       # 16 blocks
NPSUM = 8                  # psum banks rotated

_CACHE = {}


def _expm_table(W: np.ndarray) -> np.ndarray:
    """M (D, T*D) float32: columns [j*D:(j+1)*D] = expm(DT*j*W), computed in f64."""
    A = DT * W.astype(np.float64)
    # Taylor series; ||A|| ~ 0.02 so ~20 terms reach f64 machine eps.
    M1 = np.eye(D, dtype=np.float64)
    term = np.eye(D, dtype=np.float64)
    for n in range(1, 24):
        term = term @ A / n
        M1 += term
    Ms = np.empty((T, D, D), dtype=np.float64)
    Ms[0] = np.eye(D)
    for j in range(1, T):
        Ms[j] = Ms[j - 1] @ M1
    return np.ascontiguousarray(Ms.transpose(1, 0, 2).reshape(D, T * D)).astype(np.float32)


def _build_nc():
    import concourse.bass as bass
    import concourse.mybir as mybir

    f32 = mybir.dt.float32
    nc = bass.Bass(trn_type="TRN2")
    xT_d = nc.dram_tensor("xT", (D, BSH), f32, kind="ExternalInput")
    M_d = nc.dram_tensor("M", (D, T * D), f32, kind="ExternalInput")
    out_d = nc.dram_tensor("out", (BSH, T * D), f32, kind="ExternalOutput")

    xT_s = nc.alloc_sbuf_tensor("xT_s", [D, BSH], f32)
    M_s = nc.alloc_sbuf_tensor("M_s", [D, T * D], f32)
    stg = [nc.alloc_sbuf_tensor(f"stg{p}", [128, NBLK * BW], f32) for p in range(2)]
    psum = nc.alloc_psum_tensor("acc", [128, NPSUM * 512], f32)

    s_load = nc.alloc_semaphore("s_load")
    s_mm = nc.alloc_semaphore("s_mm")
    s_drain = nc.alloc_semaphore("s_drain")
    s_out = nc.alloc_semaphore("s_out")

    with nc.Block() as block:
        @block.sync
        def _(sync):
            sync.dma_start(out=xT_s[:], in_=xT_d[:]).then_inc(s_load, 16)
            for b in range(NBLK):
                sync.dma_start(out=M_s[:, b * BW:(b + 1) * BW],
                               in_=M_d[:, b * BW:(b + 1) * BW]).then_inc(s_load, 16)
            for i in range(NTILES):
                sync.wait_ge(s_drain, NBLK * (i + 1))
                sync.dma_start(out=out_d[i * 128:(i + 1) * 128, :],
                               in_=stg[i % 2][:]).then_inc(s_out, 16)
            sync.wait_ge(s_out, 16 * NTILES)

        @block.tensor
        def _(tensor):
            for i in range(NTILES):
                for b in range(NBLK):
                    k = i * NBLK + b
                    if i == 0:
                        tensor.wait_ge(s_load, 16 * (b + 2))
                    if k >= NPSUM:
                        tensor.wait_ge(s_drain, k - NPSUM + 1)
                    pb = (k % NPSUM) * 512
                    tensor.matmul(psum[:, pb:pb + 512],
                                  xT_s[:, i * 128:(i + 1) * 128],
                                  M_s[:, b * BW:(b + 1) * BW],
                                  start=True, stop=True).then_inc(s_mm, 1)

        @block.vector
        def _(vector):
            for i in range(NTILES):
                for b in range(NBLK):
                    k = i * NBLK + b
                    if b == 0 and i >= 2:
                        vector.wait_ge(s_out, 16 * (i - 1))
                    vector.wait_ge(s_mm, k + 1)
                    pb = (k % NPSUM) * 512
                    vector.tensor_copy(out=stg[i % 2][:, b * BW:(b + 1) * BW],
                                       in_=psum[:, pb:pb + 512]).then_inc(s_drain, 1)

    # Epilogue: quiesce, reset sems so a re-execution of the NEFF starts clean.
    nc.all_engine_barrier()
    nc.clear_and_free_semaphores([s_load, s_mm, s_drain, s_out])
    nc.all_engine_barrier()
    return nc


def run_on_device(x: np.ndarray, Mcat: np.ndarray, trace: bool = False):
    """Shard x across 8 cores, run, return (out (B,T,D), results_obj)."""
    from concourse.bass_utils import run_bass_kernel_spmd

    if "nc" not in _CACHE:
        _CACHE["nc"] = _build_nc()
    nc = _CACHE["nc"]

    in_maps = []
    for c in range(NCORES):
        xT_c = np.ascontiguousarray(x[c * BSH:(c + 1) * BSH].T)
        in_maps.append({"xT": xT_c, "M": Mcat})

    res = run_bass_kernel_spmd(nc, in_maps, core_ids=list(range(NCORES)), trace=trace)
    out = np.empty((B, T, D), dtype=np.float32)
    for c in range(NCORES):
        out[c * BSH:(c + 1) * BSH] = res.results[c]["out"].reshape(BSH, T, D)
    return out, res


def kernel(x, W, T):
    x = np.asarray(x, dtype=np.float32)
    W = np.asarray(W, dtype=np.float32)
    assert int(T) == 64 and x.shape == (B, D) and W.shape == (D, D)
    Mcat = _expm_table(W)
    out, _ = run_on_device(x, Mcat, trace=False)
    return out


# revision 4
# speedup vs baseline: 1.2220x; 1.2220x over previous
"""Trainium2 Bass kernel for ContinuousIntegratedKoopmanOperator.

reference: odeint(dz/dt = z @ W) sampled at t = DT*[1..T], y0 = x at t[0].
Closed form (time-invariant linear ODE): out[:, j, :] = x @ expm(DT*j*W).

Strategy:
  host: compute Mj = expm(DT*j*W) for j=0..T-1 in float64; split x and M
        into fp16 hi/lo pairs (hi + lo captures ~22 mantissa bits).
  device (8 cores, batch-sharded 1024 rows each):
        out_tile = x @ M_block via 3 accumulated full-rate fp16 matmuls
        (hi@hi + hi@lo + lo@hi; dropped lo@lo ~ 2^-22 relative).
        8 batch tiles x 16 j-blocks, PSUM bank rotation, DVE drain to
        double-buffered staging, 4MB contiguous DMA out per batch tile.
  sync: raw bass, explicit sems, one load-sem per DMA so a wait proves
        that specific transfer landed (DMA completions are unordered).
"""
import numpy as np

DT = 0.01
B, D, T = 8192, 128, 64
NCORES = 8
BSH = B // NCORES          # 1024 rows per core
NTILES = BSH // 128        # 8 batch tiles per core
BW = 512                   # j-block width (4 j's of 128)
NBLK = (T * D) // BW       # 16 blocks
NPSUM = 8                  # psum banks rotated

MODE = "fp16x3"            # "fp16x3" | "fp32"

_CACHE = {}


def _expm_table(W: np.ndarray) -> np.ndarray:
    """(D, T*D) float64: columns [j*D:(j+1)*D] = expm(DT*j*W)."""
    A = DT * W.astype(np.float64)
    M1 = np.eye(D, dtype=np.float64)
    term = np.eye(D, dtype=np.float64)
    for n in range(1, 24):
        term = term @ A / n
        M1 += term
    Ms = np.empty((T, D, D), dtype=np.float64)
    Ms[0] = np.eye(D)
    for j in range(1, T):
        Ms[j] = Ms[j - 1] @ M1
    return np.ascontiguousarray(Ms.transpose(1, 0, 2).reshape(D, T * D))


def _split16(a64: np.ndarray):
    hi = a64.astype(np.float16)
    lo = (a64 - hi.astype(np.float64)).astype(np.float16)
    return hi, lo


def _build_nc():
    import concourse.bass as bass
    import concourse.mybir as mybir

    f32 = mybir.dt.float32
    f16 = mybir.dt.float16
    fin = f16 if MODE == "fp16x3" else f32
    # per-block input column width (hi|lo pair for fp16x3)
    mw = 2 * BW if MODE == "fp16x3" else BW
    xw = 2 * BSH if MODE == "fp16x3" else BSH

    nc = bass.Bass(trn_type="TRN2")
    xT_d = nc.dram_tensor("xT", (D, xw), fin, kind="ExternalInput")
    M_d = nc.dram_tensor("M", (D, NBLK * mw), fin, kind="ExternalInput")
    out_d = nc.dram_tensor("out", (BSH, T * D), f32, kind="ExternalOutput")

    xT_s = nc.alloc_sbuf_tensor("xT_s", [D, xw], fin)
    M_s = nc.alloc_sbuf_tensor("M_s", [D, NBLK * mw], fin)
    stg = [nc.alloc_sbuf_tensor(f"stg{p}", [128, NBLK * BW], f32) for p in range(2)]
    psum = nc.alloc_psum_tensor("acc", [128, NPSUM * 512], f32)

    s_ld = [nc.alloc_semaphore(f"s_ld{i}") for i in range(1 + NBLK)]  # xT + blocks
    s_mm = nc.alloc_semaphore("s_mm")
    s_drain = nc.alloc_semaphore("s_drain")
    s_out = [nc.alloc_semaphore(f"s_out{p}") for p in range(2)]

    all_sems = [*s_ld, s_mm, s_drain, *s_out]
    nums = sorted(s.num for s in all_sems)
    assert nums == list(range(nums[0], nums[-1] + 1)), "sems not contiguous"
    sem_range = range(nums[0], nums[-1] + 1)

    # Prologue: reset sems/dma state (safe re-execution of a loaded NEFF),
    # then an NRT-level barrier so no engine uses sems before the clear.
    nc.gpsimd.dma_reset(sem_range)
    nc.gpsimd.sem_clear(sem_range)
    nc._nrt_pseudo_barrier()

    def mm_triple(tensor, i, b, pb, first_sync=None):
        """Emit the matmul(s) for (batch tile i, block b) into psum[:, pb:pb+512]."""
        if MODE == "fp16x3":
            x_hi = xT_s[:, i * 128:(i + 1) * 128]
            x_lo = xT_s[:, BSH + i * 128:BSH + (i + 1) * 128]
            m_hi = M_s[:, b * mw:b * mw + BW]
            m_lo = M_s[:, b * mw + BW:b * mw + 2 * BW]
            tensor.matmul(psum[:, pb:pb + 512], x_hi, m_hi, start=True, stop=False)
            tensor.matmul(psum[:, pb:pb + 512], x_hi, m_lo, start=False, stop=False)
            return tensor.matmul(psum[:, pb:pb + 512], x_lo, m_hi,
                                 start=False, stop=True)
        else:
            return tensor.matmul(psum[:, pb:pb + 512],
                                 xT_s[:, i * 128:(i + 1) * 128],
                                 M_s[:, b * BW:(b + 1) * BW],
                                 start=True, stop=True)

    with nc.Block() as block:
        @block.sync
        def _(sync):
            sync.dma_start(out=xT_s[:], in_=xT_d[:]).then_inc(s_ld[0], 16)
            for b in range(NBLK):
                sync.dma_start(out=M_s[:, b * mw:(b + 1) * mw],
                               in_=M_d[:, b * mw:(b + 1) * mw]).then_inc(s_ld[1 + b], 16)
            for i in range(NTILES):
                sync.wait_ge(s_drain, NBLK * (i + 1))
                sync.dma_start(out=out_d[i * 128:(i + 1) * 128, :],
                               in_=stg[i % 2][:]).then_inc(s_out[i % 2], 16)
            sync.wait_ge(s_out[0], 16 * (NTILES // 2))
            sync.wait_ge(s_out[1], 16 * (NTILES // 2))

        @block.tensor
        def _(tensor):
            for i in range(NTILES):
                for b in range(NBLK):
                    k = i * NBLK + b
                    if i == 0:
                        if b == 0:
                            tensor.wait_ge(s_ld[0], 16)
                        tensor.wait_ge(s_ld[1 + b], 16)
                    if k >= NPSUM:
                        tensor.wait_ge(s_drain, k - NPSUM + 1)
                    pb = (k % NPSUM) * 512
                    mm_triple(tensor, i, b, pb).then_inc(s_mm, 1)

        @block.vector
        def _(vector):
            for i in range(NTILES):
                for b in range(NBLK):
                    k = i * NBLK + b
                    if b == 0 and i >= 2:
                        vector.wait_ge(s_out[i % 2], 16 * (i // 2))
                    vector.wait_ge(s_mm, k + 1)
                    pb = (k % NPSUM) * 512
                    vector.tensor_copy(out=stg[i % 2][:, b * BW:(b + 1) * BW],
                                       in_=psum[:, pb:pb + 512]).then_inc(s_drain, 1)

    return nc


def _prep_inputs(x: np.ndarray, Mcat64: np.ndarray):
    """Per-core input maps. Mcat64 is the (D, T*D) float64 expm table."""
    if MODE == "fp16x3":
        M_hi, M_lo = _split16(Mcat64)
        # interleave per block: [hi_b | lo_b] pairs
        Mb = np.empty((D, NBLK * 2 * BW), dtype=np.float16)
        for b in range(NBLK):
            Mb[:, b * 2 * BW:b * 2 * BW + BW] = M_hi[:, b * BW:(b + 1) * BW]
            Mb[:, b * 2 * BW + BW:(b + 1) * 2 * BW] = M_lo[:, b * BW:(b + 1) * BW]
        maps = []
        for c in range(NCORES):
            xT = x[c * BSH:(c + 1) * BSH].T.astype(np.float64)
            x_hi, x_lo = _split16(xT)
            xc = np.concatenate([x_hi, x_lo], axis=1)
            maps.append({"xT": np.ascontiguousarray(xc), "M": Mb})
        return maps
    else:
        Mf = Mcat64.astype(np.float32)
        return [{"xT": np.ascontiguousarray(x[c * BSH:(c + 1) * BSH].T), "M": Mf}
                for c in range(NCORES)]


def run_on_device(x: np.ndarray, Mcat64: np.ndarray, trace: bool = False):
    from concourse.bass_utils import run_bass_kernel_spmd

    if "nc" not in _CACHE:
        _CACHE["nc"] = _build_nc()
    nc = _CACHE["nc"]

    in_maps = _prep_inputs(x, Mcat64)
    res = run_bass_kernel_spmd(nc, in_maps, core_ids=list(range(NCORES)), trace=trace)
    out = np.empty((B, T, D), dtype=np.float32)
    for c in range(NCORES):
        out[c * BSH:(c + 1) * BSH] = res.results[c]["out"].reshape(BSH, T, D)
    return out, res


def kernel(x, W, T):
    x = np.asarray(x, dtype=np.float32)
    W = np.asarray(W, dtype=np.float32)
    assert int(T) == 64 and x.shape == (B, D) and W.shape == (D, D)
    Mcat64 = _expm_table(W)
    out, _ = run_on_device(x, Mcat64, trace=False)
    return out
